# revision 5
# baseline (speedup 1.0000x reference)
"""DYConv2d (dynamic conv with rank-1 4D attention) on 8 Trainium2 cores.

Data-parallel over batch: each core takes 4 samples, synthesizes its
per-sample conv weights on device, and runs the per-sample 3x3 conv as
implicit GEMM (9 shifted matmuls x 2 C-halves accumulated in PSUM).

Self-contained: hardcodes all shapes; host side only reshapes/pads
inputs, shards across cores, and concatenates the per-core outputs.
"""

import numpy as np

B, C, O, KS, H, W, R = 32, 256, 256, 3, 56, 56, 16
KK = KS * KS  # 9
NCORES = 8
BL = B // NCORES  # 4 samples per core
WP = W + 2  # 58 (host-padded width)
HP = H + 2  # 58 (vertical pad rows live only in SBUF)
NPIX = HP * WP  # 3364
RG = 7  # row groups per image
RGH = 8  # output rows per group
NT = RGH * W  # 448 matmul free dim (<=512 fp32 PSUM bank)

TRACE = False
LAST_EXEC_NS = None
LAST_RESULTS = None

_CACHED = None


def _build_program():
    """Build + compile the per-core Bass program (cached)."""
    global _CACHED
    if _CACHED is not None:
        return _CACHED

    from contextlib import ExitStack

    from concourse import bacc
    import concourse.mybir as mybir
    import concourse.tile as tile

    f32 = mybir.dt.float32
    f32r = mybir.dt.float32r
    AF = mybir.ActivationFunctionType
    AX = mybir.AxisListType

    nc = bacc.Bacc("TRN2", target_bir_lowering=False, debug=False)

    x_d = nc.dram_tensor("x", [BL, C, HP, WP], f32r, kind="ExternalInput").ap()
    bwT_d = nc.dram_tensor("bwT", [2, 128, KK * O], f32, kind="ExternalInput").ap()
    fcsh_d = nc.dram_tensor("fcsh", [2, 128, R], f32, kind="ExternalInput").ap()
    bsh_d = nc.dram_tensor("bsh", [R, 1], f32, kind="ExternalInput").ap()
    fcinT_d = nc.dram_tensor("fcinT", [R + 1, C], f32, kind="ExternalInput").ap()
    fcoupT_d = nc.dram_tensor("fcoupT", [R + 1, O], f32, kind="ExternalInput").ap()
    fckT_d = nc.dram_tensor("fckT", [R + 1, KK], f32, kind="ExternalInput").ap()
    out_d = nc.dram_tensor("out", [BL, O, H, W], f32, kind="ExternalOutput").ap()

    with tile.TileContext(nc) as tc, ExitStack() as ctx:
        persist = ctx.enter_context(tc.tile_pool(name="persist", bufs=1))
        conv_psum = ctx.enter_context(
            tc.tile_pool(name="conv_psum", bufs=4, space="PSUM")
        )
        attn_psum = ctx.enter_context(
            tc.tile_pool(name="attn_psum", bufs=3, space="PSUM")
        )
        out_pool = ctx.enter_context(tc.tile_pool(name="out_pool", bufs=4))

        # ---- static weights (loaded once) ----
        bwT_sb = []
        fcsh_sb = []
        for ct in range(2):
            t = persist.tile([128, KK * O], f32, tag=f"bwT{ct}", name=f"bwT{ct}")
            nc.sync.dma_start(t[:], bwT_d[ct])
            bwT_sb.append(t)
            t = persist.tile([128, R], f32, tag=f"fcsh{ct}", name=f"fcsh{ct}")
            nc.sync.dma_start(t[:], fcsh_d[ct])
            fcsh_sb.append(t)
        bsh_sb = persist.tile([R, 1], f32, tag="bsh", name="bsh_sb")
        nc.sync.dma_start(bsh_sb[:], bsh_d[:])
        fcinT_sb = persist.tile([R + 1, C], f32, tag="fcinT", name="fcinT_sb")
        nc.sync.dma_start(fcinT_sb[:], fcinT_d[:])
        fcoupT_sb = persist.tile([R + 1, O], f32, tag="fcoupT", name="fcoupT_sb")
        nc.sync.dma_start(fcoupT_sb[:], fcoupT_d[:])
        fckT_sb = persist.tile([R + 1, KK], f32, tag="fckT", name="fckT_sb")
        nc.sync.dma_start(fckT_sb[:], fckT_d[:])

        # ---- double-buffered per-sample state (slot = b % 2) ----
        ximg, ximg_v, w_sb, s_col, colsc_sb = [], [], [], [], []
        h_ext, ainp_row, ak_row, aoup_sb = [], [], [], []
        for s in range(2):
            ximg.append([persist.tile([128, NPIX], f32r, tag=f"ximg{s}{ct}", name=f"ximg{s}{ct}") for ct in range(2)])
            ximg_v.append([t[:].rearrange("p (r c) -> p r c", r=HP) for t in ximg[s]])
            w_sb.append([persist.tile([128, KK * O], f32r, tag=f"wsb{s}{ct}", name=f"wsb{s}{ct}") for ct in range(2)])
            s_col.append([persist.tile([128, 1], f32, tag=f"scol{s}{ct}", name=f"scol{s}{ct}") for ct in range(2)])
            colsc_sb.append([persist.tile([128, KK], f32, tag=f"colsc{s}{ct}", name=f"colsc{s}{ct}") for ct in range(2)])
            h_ext.append(persist.tile([R + 1, 1], f32, tag=f"hext{s}", name=f"hext{s}"))
            ainp_row.append(persist.tile([1, C], f32, tag=f"ainp{s}", name=f"ainp{s}"))
            ak_row.append(persist.tile([1, KK], f32, tag=f"ak{s}", name=f"akrow{s}"))
            aoup_sb.append(persist.tile([128, 2], f32, tag=f"aoup{s}", name=f"aoup{s}"))
            # ones everywhere; rows 0..R-1 get overwritten by the relu
            nc.vector.memset(h_ext[s][:], 1.0)

        def emit_img_dma(s, b):
            for ct in range(2):
                nc.sync.dma_start(
                    ximg_v[s][ct][:, :, :],
                    x_d[b, ct * 128 : (ct + 1) * 128],
                )

        def emit_stage_b(s, b):
            # per-channel sums (pad zeros don't affect them) -> h = relu(...)
            for ct in range(2):
                nc.vector.reduce_sum(s_col[s][ct][:], ximg[s][ct][:].bitcast(f32), axis=AX.X)
            hp = attn_psum.tile([R, 1], f32, tag="apsum", name="hp")
            nc.tensor.matmul(hp[:], fcsh_sb[0][:], s_col[s][0][:], start=True, stop=False)
            nc.tensor.matmul(hp[:], fcsh_sb[1][:], s_col[s][1][:], start=False, stop=True)
            nc.scalar.activation(h_ext[s][0:R, :], hp[:], AF.Relu, bias=bsh_sb[:])

        def emit_stage_c(s, b):
            ainp_p = attn_psum.tile([1, C], f32, tag="apsum", name="ainp_p")
            nc.tensor.matmul(ainp_p[:], h_ext[s][:], fcinT_sb[:], start=True, stop=True)
            nc.scalar.activation(ainp_row[s][:], ainp_p[:], AF.Sigmoid)
            ak_p = attn_psum.tile([1, KK], f32, tag="apsum", name="ak_p")
            nc.tensor.matmul(ak_p[:], h_ext[s][:], fckT_sb[:], start=True, stop=True)
            nc.scalar.activation(ak_row[s][:], ak_p[:], AF.Sigmoid)
            for ot in range(2):
                ao_p = attn_psum.tile([128, 1], f32, tag="apsum", name="ao_p")
                nc.tensor.matmul(
                    ao_p[:],
                    fcoupT_sb[:, ot * 128 : (ot + 1) * 128],
                    h_ext[s][:],
                    start=True,
                    stop=True,
                )
                nc.scalar.activation(aoup_sb[s][:, ot : ot + 1], ao_p[:], AF.Sigmoid)

        def emit_stage_d(s, b):
            # colsc[c, k] = a_inp[c] * a_k[k] (rank-1 outer product on PE)
            for ct in range(2):
                cs_p = attn_psum.tile([128, KK], f32, tag="apsum", name="cs_p")
                nc.tensor.matmul(
                    cs_p[:],
                    ainp_row[s][:, ct * 128 : (ct + 1) * 128],
                    ak_row[s][:],
                    start=True,
                    stop=True,
                )
                nc.scalar.activation(colsc_sb[s][ct][:], cs_p[:], AF.Copy)

        def emit_stage_e(s, b):
            # w[c, k*O+o] = base_wT[c, k*O+o] * colsc[c, k]
            for ct in range(2):
                nc.vector.tensor_mul(
                    w_sb[s][ct][:].rearrange("p (k o) -> p k o", k=KK),
                    bwT_sb[ct][:].rearrange("p (k o) -> p k o", k=KK),
                    colsc_sb[s][ct][:, :, None].broadcast_to((128, KK, O)),
                )

        def emit_conv_group(s, b, ot, rg):
            ps = conv_psum.tile([128, NT], f32, tag="cpsum", name="cps")
            first = True
            for ct in range(2):
                for kh in range(KS):
                    for kw in range(KS):
                        k = kh * KS + kw
                        nc.tensor.matmul(
                            ps[:],
                            w_sb[s][ct][
                                :, k * O + ot * 128 : k * O + ot * 128 + 128
                            ],
                            ximg_v[s][ct][
                                :, rg * RGH + kh : rg * RGH + kh + RGH, kw : kw + W
                            ],
                            start=first,
                            stop=(ct == 1 and k == KK - 1),
                        )
                        first = False
            osb = out_pool.tile([128, NT], f32, tag="osb", name="osb")
            nc.scalar.activation(
                osb[:], ps[:], AF.Copy, scale=aoup_sb[s][:, ot : ot + 1]
            )
            nc.sync.dma_start(
                out_d[b, ot * 128 : (ot + 1) * 128, rg * RGH : (rg + 1) * RGH, :],
                osb[:].rearrange("p (r c) -> p r c", r=RGH),
            )

        # ---- pipeline ----
        emit_img_dma(0, 0)
        emit_stage_b(0, 0)
        emit_stage_c(0, 0)
        emit_stage_d(0, 0)
        emit_stage_e(0, 0)
        for b in range(BL):
            s = b % 2
            sn = (b + 1) % 2
            gi = 0
            for ot in range(2):
                for rg in range(RG):
                    emit_conv_group(s, b, ot, rg)
                    gi += 1
                    if b + 1 < BL:
                        if gi == 1:
                            emit_img_dma(sn, b + 1)
                        elif gi == 5:
                            emit_stage_b(sn, b + 1)
                        elif gi == 8:
                            emit_stage_c(sn, b + 1)
                        elif gi == 10:
                            emit_stage_d(sn, b + 1)
                        elif gi == 12:
                            emit_stage_e(sn, b + 1)

    nc.compile()
    _CACHED = nc
    return nc


def kernel(x, base_w, fc_share_w, fc_share_b, fc_inp_w, fc_inp_b,
           fc_oup_w, fc_oup_b, fc_k_w, fc_k_b):
    global LAST_EXEC_NS, LAST_RESULTS
    from concourse.bass_utils import run_bass_kernel_spmd

    nc = _build_program()

    x = np.asarray(x, np.float32)
    # host-side zero pad of H and W: every image DMA is one contiguous
    # chunk per partition and refreshes the pad border on each load
    xp = np.zeros((B, C, HP, WP), np.float32)
    xp[:, :, 1 : H + 1, 1 : W + 1] = x

    bwT = np.ascontiguousarray(
        np.asarray(base_w, np.float32).transpose(1, 2, 3, 0).reshape(2, 128, KK * O)
    )
    fcsh = np.ascontiguousarray(
        (np.asarray(fc_share_w, np.float32) / float(H * W)).T.reshape(2, 128, R)
    )
    bsh = np.ascontiguousarray(np.asarray(fc_share_b, np.float32).reshape(R, 1))
    fcinT = np.ascontiguousarray(
        np.concatenate([np.asarray(fc_inp_w, np.float32).T,
                        np.asarray(fc_inp_b, np.float32)[None, :]], axis=0)
    )
    fcoupT = np.ascontiguousarray(
        np.concatenate([np.asarray(fc_oup_w, np.float32).T,
                        np.asarray(fc_oup_b, np.float32)[None, :]], axis=0)
    )
    fckT = np.ascontiguousarray(
        np.concatenate([np.asarray(fc_k_w, np.float32).T,
                        np.asarray(fc_k_b, np.float32)[None, :]], axis=0)
    )

    in_maps = []
    for i in range(NCORES):
        in_maps.append(
            {
                "x": np.ascontiguousarray(xp[i * BL : (i + 1) * BL]),
                "bwT": bwT,
                "fcsh": fcsh,
                "bsh": bsh,
                "fcinT": fcinT,
                "fcoupT": fcoupT,
                "fckT": fckT,
            }
        )

    res = run_bass_kernel_spmd(nc, in_maps, list(range(NCORES)), trace=TRACE)
    LAST_EXEC_NS = res.exec_time_ns
    LAST_RESULTS = res
    return np.concatenate([res.results[i]["out"] for i in range(NCORES)], axis=0)
